# revision 28
# baseline (speedup 1.0000x reference)
"""Self-contained Trainium2 Bass kernel for nn_GAT (GNN message passing).

Layout (per core, SPMD across NCORES cores):
  - Nodes sharded by graph id; within a core nodes are sorted by
    max(degA, degB) desc and padded to sp_data rows (mult of 128).
  - Global node table row = m*sp_tab + local_row where sp_tab = sp_data+128;
    the extra 128 rows per core are an all-pad tile whose a_src is -inf so
    that padded edge slots contribute exp(-inf)=0 to the segment softmax
    (no mask tensor needed).
  - Table rows are [h(64 bf16) | a_src(4 f32 as raw bits) | junk] = 256B,
    rebuilt per layer via local matmul + AllGather.
  - Edges sharded by dst owner, laid out node-major: dst local row ->
    (tile, partition); slots padded to uniform L per (group, A|B) where
    A = src table row < T0_ROWS (= cores 0..NCORES/2-1).
  - Groups pack consecutive tiles greedily while tig*(L0+L1) <= CCAP.
"""

import numpy as np

HID = 16
HEADS = 4
F_HID = HID * HEADS  # 64
NEG_SLOPE = 0.2
EPS = 1e-16
P = 128
ROW_F = 128  # bf16 per table row (256B)


def build_plan(x, edge_index, batch, ng=128, ncores=8, ccap=64, tigmax=8):
    x = np.asarray(x, np.float32)
    ei = np.asarray(edge_index, np.int64)
    batch = np.asarray(batch, np.int64)
    N, f_in = x.shape
    g_per_core = ng // ncores

    # self-loops are NOT gathered: each node's own row is copied from the
    # local slice kept in SBUF into a dedicated slot column per tile
    src = ei[0]
    dst = ei[1]

    node_core = batch // g_per_core
    counts = np.bincount(node_core, minlength=ncores)
    sp_data = int(np.ceil(counts.max() / P) * P)
    n_tiles = sp_data // P
    sp_tab = sp_data + P  # one extra all-pad tile per core (a_src = -inf)
    Npad = ncores * sp_tab
    T0C = ncores // 2
    T0_ROWS = T0C * sp_tab
    assert T0_ROWS < 32768 and (Npad - T0_ROWS) < 32768, (T0_ROWS, Npad)
    padrowA = sp_data  # core 0's pad tile, global row sp_data < T0_ROWS
    padrowB = T0C * sp_tab + sp_data - T0_ROWS  # core T0C's pad tile, local
    core_start = np.concatenate([[0], np.cumsum(counts)])

    src_core = node_core[src]
    isA = src_core < T0C
    degA = np.bincount(dst, weights=isA.astype(np.float64), minlength=N).astype(np.int64)
    degB = np.bincount(dst, weights=(~isA).astype(np.float64), minlength=N).astype(np.int64)

    # within-core permutation: band by max(degA, degB) desc; inside a band
    # cluster A-heavy and B-heavy nodes apart (lowers per-tile maxA+maxB)
    glob_row = np.empty(N, np.int64)
    row_node = np.full(ncores * sp_tab, -1, np.int64)
    for m in range(ncores):
        lo, hi = core_start[m], core_start[m + 1]
        nodes = np.arange(lo, hi)
        key = (
            np.maximum(degA[nodes], degB[nodes]) * 1000000
            + (degA[nodes] >= degB[nodes]) * 500000
            + np.minimum(degA[nodes], degB[nodes])
        )
        nodes = nodes[np.argsort(-key, kind="stable")]
        glob_row[nodes] = m * sp_tab + np.arange(len(nodes))
        row_node[m * sp_tab + np.arange(len(nodes))] = nodes

    # per-tile max degrees across cores
    tile_degA = np.zeros((ncores, n_tiles), np.int64)
    tile_degB = np.zeros((ncores, n_tiles), np.int64)
    for m in range(ncores):
        rows = np.arange(counts[m])
        nodes = row_node[m * sp_tab + rows]
        t = rows // P
        np.maximum.at(tile_degA[m], t, degA[nodes])
        np.maximum.at(tile_degB[m], t, degB[nodes])
    gmaxA = np.maximum(tile_degA.max(axis=0), 1)
    gmaxB = np.maximum(tile_degB.max(axis=0), 1)

    # greedy group packing: while tig*(max L0 + max L1) <= ccap
    groups = []
    t = 0
    while t < n_tiles:
        tig = 1
        while (
            t + tig < n_tiles
            and tig < tigmax
            and (tig + 1)
            * (max(gmaxA[t : t + tig + 1]) + max(gmaxB[t : t + tig + 1]))
            <= ccap
        ):
            tig += 1
        L0 = int(max(gmaxA[t : t + tig]))
        L1 = int(max(gmaxB[t : t + tig]))
        groups.append(dict(base=t, tig=tig, L0=L0, L1=L1))
        t += tig

    # offsets in the concatenated idx inputs
    offA = offB = 0
    for g in groups:
        cA, cB = g["tig"] * g["L0"], g["tig"] * g["L1"]
        g["offA"], g["wA"] = offA, cA * 8  # cA*128 idxs / 16 rows
        g["offB"], g["wB"] = offB, cB * 8
        offA += g["wA"]
        offB += g["wB"]
    WA, WB = offA, offB

    # ---- per-core edge slot assignment ----
    dst_owner = node_core[dst]
    loc_row = glob_row[dst] - dst_owner * sp_tab
    src_row = glob_row[src]

    order = np.lexsort((~isA, loc_row, dst_owner))
    so_owner = dst_owner[order]
    so_loc = loc_row[order]
    so_isA = isA[order]
    so_srcrow = src_row[order]
    key = so_owner * (Npad * 2) + so_loc * 2 + (~so_isA).astype(np.int64)
    newrun = np.concatenate([[True], key[1:] != key[:-1]])
    run_start = np.flatnonzero(newrun)
    slot = np.arange(len(key)) - run_start[np.cumsum(newrun) - 1]

    tile_of = so_loc // P
    part_of = so_loc % P
    group_of = np.zeros(n_tiles, np.int64)
    for gi, g in enumerate(groups):
        group_of[g["base"] : g["base"] + g["tig"]] = gi

    idxA_cat, idxB_cat = [], []
    for m in range(ncores):
        emask = so_owner == m
        et = tile_of[emask]
        ep = part_of[emask]
        eA = so_isA[emask]
        esrc = so_srcrow[emask]
        eslot = slot[emask]
        eg = group_of[et]

        iA_full = np.full((16, WA), padrowA, np.int16)
        iB_full = np.full((16, WB), padrowB, np.int16)
        for gi, g in enumerate(groups):
            tig, l0, l1 = g["tig"], g["L0"], g["L1"]
            gselA = (eg == gi) & eA
            gselB = (eg == gi) & ~eA
            tt = et - g["base"]
            cidx = tt[gselA] * l0 + eslot[gselA]
            q = cidx * P + ep[gselA]
            iA_full[q % 16, g["offA"] + q // 16] = esrc[gselA].astype(np.int16)
            cidx = tt[gselB] * l1 + eslot[gselB]
            q = cidx * P + ep[gselB]
            iB_full[q % 16, g["offB"] + q // 16] = (esrc[gselB] - T0_ROWS).astype(
                np.int16
            )
            assert esrc[gselA].max(initial=0) < T0_ROWS
            assert (esrc[gselB] - T0_ROWS).max(initial=0) < Npad - T0_ROWS
        idxA_cat.append(iA_full)
        idxB_cat.append(iB_full)

    # ---- xT per core [f_in, sp_data] ----
    xT = []
    for m in range(ncores):
        xs = np.zeros((sp_data, f_in), np.float32)
        nodes = row_node[m * sp_tab : m * sp_tab + counts[m]]
        xs[: counts[m]] = x[nodes]
        xT.append(np.ascontiguousarray(xs.T))

    # ---- selectors [128, n_tiles*g_per_core] per core ----
    sel = []
    for m in range(ncores):
        s = np.zeros((P, n_tiles, g_per_core), np.float32)
        rows = np.arange(counts[m])
        nodes = row_node[m * sp_tab + rows]
        s[rows % P, rows // P, batch[nodes] - m * g_per_core] = 1.0
        sel.append(s.reshape(P, n_tiles * g_per_core))

    struct = dict(
        sp_data=sp_data,
        sp_tab=sp_tab,
        n_tiles=n_tiles,
        Npad=Npad,
        T0_ROWS=T0_ROWS,
        groups=groups,
        WA=WA,
        WB=WB,
        f_in=f_in,
        g_per_core=g_per_core,
        ncls=None,  # fill in later
        ncores=ncores,
        ng=ng,
    )
    glob = dict(glob_row=glob_row, row_node=row_node, counts=counts)
    percore = dict(idxA=idxA_cat, idxB=idxB_cat, xT=xT, sel=sel)
    return struct, percore, glob


def augment_weights(W, a_s, a_d):
    W = np.asarray(W, np.float32)
    a_s = np.asarray(a_s, np.float32)
    a_d = np.asarray(a_d, np.float32)
    As = np.zeros((F_HID, HEADS), np.float32)
    Ad = np.zeros((F_HID, HEADS), np.float32)
    for h in range(HEADS):
        As[h * HID : (h + 1) * HID, h] = a_s[h]
        Ad[h * HID : (h + 1) * HID, h] = a_d[h]
    return np.concatenate([W, W @ As, W @ Ad], axis=1).astype(np.float32)


def make_inmaps(inputs, struct, percore, layers=3):
    """Build the per-core input dicts for run_bass_kernel_spmd."""
    import ml_dtypes

    bf16 = ml_dtypes.bfloat16
    ncores = struct["ncores"]
    ws = [
        augment_weights(inputs[f"W{l}"], inputs[f"as{l}"], inputs[f"ad{l}"])
        for l in range(layers)
    ]
    biases = np.concatenate(
        [np.asarray(inputs[f"b{l}"], np.float32) for l in range(layers)]
    )
    bias_rep = np.tile(biases[None, :], (P, 1))
    wr = np.asarray(inputs["Wr"], np.float32)
    br_rep = np.tile(np.asarray(inputs["br"], np.float32)[None, :], (P, 1))
    in_maps = []
    for m in range(ncores):
        im = dict(
            xT=percore["xT"][m].astype(bf16),
            idxA=percore["idxA"][m],
            idxB=percore["idxB"][m],
            sel=percore["sel"][m].astype(bf16),
            biases=bias_rep,
            wr=wr,
            br=br_rep,
        )
        for l in range(layers):
            im[f"w{l}aug"] = ws[l].astype(bf16)
        in_maps.append(im)
    return in_maps


def numpy_model(inputs, struct, percore, glob, layers=3):
    """Numpy re-implementation of the device algorithm (same padded layout)."""
    spd = struct["sp_data"]
    spt = struct["sp_tab"]
    Npad = struct["Npad"]
    T0 = struct["T0_ROWS"]
    ncores = struct["ncores"]
    gpc = struct["g_per_core"]
    nt = struct["n_tiles"]
    ncls = np.asarray(inputs["Wr"]).shape[1]
    batch = np.asarray(inputs["batch"], np.int64)

    ws = [
        augment_weights(inputs[f"W{l}"], inputs[f"as{l}"], inputs[f"ad{l}"])
        for l in range(layers)
    ]
    biases = [np.asarray(inputs[f"b{l}"], np.float32) for l in range(layers)]

    acts = [None] * ncores
    out_logits = np.zeros((struct["ng"], ncls), np.float32)

    for layer in range(layers):
        # table: h (64) + a_src (4) per row; pad tile rows have a_src=-inf
        tb_h = np.zeros((Npad, F_HID), np.float32)
        tb_a = np.full((Npad, HEADS), 0.0, np.float32)
        for m in range(ncores):
            a = percore["xT"][m].T.astype(np.float32) if layer == 0 else acts[m]
            hw = a @ ws[layer]
            tb_h[m * spt : m * spt + spd] = hw[:, :64]
            tb_a[m * spt : m * spt + spd] = hw[:, 64:68]
            tb_a[m * spt + spd : (m + 1) * spt] = -np.inf
        for m in range(ncores):
            out = np.zeros((spd, F_HID), np.float32)
            a = percore["xT"][m].T.astype(np.float32) if layer == 0 else acts[m]
            a_dst_loc = (a @ ws[layer])[:, 68:72]
            for g in struct["groups"]:
                tig, l0, l1 = g["tig"], g["L0"], g["L1"]
                cA, cB = tig * l0, tig * l1
                C = tig + cA + cB  # [self | A | B] slot columns
                iw = percore["idxA"][m][:16, g["offA"] : g["offA"] + g["wA"]]
                iA = iw.T.reshape(-1)[: cA * P].astype(np.int64)
                iw = percore["idxB"][m][:16, g["offB"] : g["offB"] + g["wB"]]
                iB = iw.T.reshape(-1)[: cB * P].astype(np.int64)
                # self columns: local rows of this group's tiles
                srow = (
                    m * spt
                    + ((g["base"] + np.arange(tig)) * P)[None, :]
                    + np.arange(P)[:, None]
                )  # [P, tig]
                GHS = tb_h[srow]  # [P, tig, 64]
                GAS = tb_a[srow]  # [P, tig, 4]
                GHA = tb_h[:T0][iA].reshape(cA, P, F_HID).transpose(1, 0, 2)
                GHB = tb_h[T0:][iB].reshape(cB, P, F_HID).transpose(1, 0, 2)
                GH = np.concatenate([GHS, GHA, GHB], axis=1)  # [P, C, 64]
                GA_ = tb_a[:T0][iA].reshape(cA, P, HEADS).transpose(1, 0, 2)
                GB_ = tb_a[T0:][iB].reshape(cB, P, HEADS).transpose(1, 0, 2)
                GA = np.concatenate([GAS, GA_, GB_], axis=1)  # [P, C, 4]
                tt = np.concatenate(
                    [
                        np.arange(tig),
                        np.repeat(np.arange(tig), l0),
                        np.repeat(np.arange(tig), l1),
                    ]
                )
                node_rows = (g["base"] + tt)[None, :] * P + np.arange(P)[:, None]
                a_d = a_dst_loc[node_rows]  # [128, C, 4]
                logits = GA + a_d
                logits = np.where(logits >= 0, logits, NEG_SLOPE * logits)
                e = np.exp(logits)  # pad slots: exp(-inf) = 0
                s = np.zeros((P, tig, HEADS), np.float32)
                np.add.at(s, (slice(None), tt), e)
                alpha = e * (1.0 / (s + EPS))[:, tt]
                msg = GH.reshape(P, C, HEADS, HID) * alpha[:, :, :, None]
                acc = np.zeros((P, tig, F_HID), np.float32)
                np.add.at(acc, (slice(None), tt), msg.reshape(P, C, F_HID))
                for t in range(tig):
                    out[(g["base"] + t) * P + np.arange(P)] = acc[:, t]
            act = out + biases[layer][None, :]
            act = act * (1.0 / (1.0 + np.exp(-act)))
            acts[m] = act.astype(np.float32)

    for m in range(ncores):
        sel = percore["sel"][m].reshape(P, nt, gpc)
        a = acts[m].reshape(nt, P, F_HID)
        pooled = np.einsum("ptg,tpf->gf", sel, a)
        lg = pooled @ np.asarray(inputs["Wr"]) + np.asarray(inputs["br"])
        lg = np.maximum(lg, 0.0)
        mxv = lg.max(axis=1, keepdims=True)
        ls = lg - mxv - np.log(np.exp(lg - mxv).sum(axis=1, keepdims=True))
        out_logits[m * gpc : (m + 1) * gpc] = ls
    return out_logits


# ======== Bass/Tile kernel builder (8 NeuronCores) ========

from contextlib import ExitStack

import concourse.bass as bass
import concourse.tile as tile
from concourse import bacc
from concourse import mybir
from concourse.library_config import mlp as mlp_lib
from concourse.masks import make_identity

F32 = mybir.dt.float32
BF16 = mybir.dt.bfloat16
I16 = mybir.dt.int16
AF = mybir.ActivationFunctionType
OP = mybir.AluOpType


def build_gat(S, n_cores=8):
    """S: static plan dict (see build_plan). Returns nc."""
    import os

    dbg_layers = int(os.environ.get("GAT_NLAYERS", "3"))
    dbg_edge = int(os.environ.get("GAT_EDGE", "1"))
    dbg_ngroups = int(os.environ.get("GAT_NGROUPS", "999"))
    GCH = int(os.environ.get("GAT_GCH", "8"))  # >8 overflows the SWDGE ring
    tbl_space = os.environ.get("GAT_TBL", "Shared")
    spd = S["sp_data"]
    spt = S["sp_tab"]
    nt = S["n_tiles"]
    Npad = S["Npad"]
    T0 = S["T0_ROWS"]
    groups = S["groups"]
    WA = S["WA"]
    WB = S["WB"]
    GPC = S["g_per_core"]  # graphs per core (16)
    NCLS = S["ncls"]
    HEADS, HID = 4, 16
    FH = HEADS * HID  # 64
    NEG = 0.2
    LAYERS = 3
    in_dims = [S["f_in"], FH, FH]

    nc = bacc.Bacc("TRN2", debug=False, num_devices=n_cores, num_swdge_queues=4)

    # ---------------- I/O ----------------
    xT_d = nc.dram_tensor("xT", [in_dims[0], spd], BF16, kind="ExternalInput")
    idxA_d = nc.dram_tensor("idxA", [16, WA], I16, kind="ExternalInput")
    idxB_d = nc.dram_tensor("idxB", [16, WB], I16, kind="ExternalInput")
    sel_d = nc.dram_tensor("sel", [P, nt * GPC], BF16, kind="ExternalInput")
    w_d = [
        nc.dram_tensor(f"w{l}aug", [in_dims[l], 72], BF16, kind="ExternalInput")
        for l in range(LAYERS)
    ]
    bias_d = nc.dram_tensor("biases", [P, LAYERS * FH], F32, kind="ExternalInput")
    wr_d = nc.dram_tensor("wr", [FH, NCLS], F32, kind="ExternalInput")
    br_d = nc.dram_tensor("br", [P, NCLS], F32, kind="ExternalInput")
    out_d = nc.dram_tensor("out", [GPC, NCLS], F32, kind="ExternalOutput")

    slice_d = [nc.dram_tensor(f"slice{l}", [spt, P], BF16) for l in range(LAYERS)]
    table_d = [
        nc.dram_tensor(f"table{l}", [Npad, P], BF16, addr_space=tbl_space)
        for l in range(LAYERS)
    ]

    rg = [list(range(n_cores))]
    Cmax = max(g["tig"] * (1 + g["L0"] + g["L1"]) for g in groups)
    tigmax = max(g["tig"] for g in groups)

    with tile.TileContext(nc) as tc, ExitStack() as ctx:
        pers = ctx.enter_context(tc.tile_pool(name="pers", bufs=1))
        gpool = ctx.enter_context(tc.tile_pool(name="G", bufs=5))
        ltpool = ctx.enter_context(tc.tile_pool(name="lt", bufs=3))
        stat = ctx.enter_context(tc.tile_pool(name="stat", bufs=3))
        opool = ctx.enter_context(tc.tile_pool(name="oacc", bufs=3))
        rowp = ctx.enter_context(tc.tile_pool(name="row", bufs=3))
        psum = ctx.enter_context(tc.tile_pool(name="psum", bufs=3, space="PSUM"))
        psumT = ctx.enter_context(tc.tile_pool(name="psumT", bufs=2, space="PSUM"))
        psumP = ctx.enter_context(tc.tile_pool(name="psumP", bufs=1, space="PSUM"))

        # ---- persistent SBUF ----
        xT_sb = pers.tile([in_dims[0], spd], BF16)
        sel_sb = pers.tile([P, nt * GPC], BF16)
        idxA_sb = pers.tile([P, WA], I16)
        idxB_sb = pers.tile([P, WB], I16)
        w_sb = [
            pers.tile([in_dims[l], 72], BF16, name=f"w{l}sb", tag=f"w{l}sb")
            for l in range(LAYERS)
        ]
        bias_sb = pers.tile([P, LAYERS * FH], F32)
        wr_sb = pers.tile([FH, NCLS], F32)
        br_sb = pers.tile([P, NCLS], F32)
        adst_sb = [pers.tile([P, nt * HEADS], F32, name=f"adst{i}") for i in range(2)]
        slice_sb = pers.tile([P, nt * P], BF16)  # local slice rows (self-loop source)
        out_sb = pers.tile([P, nt * FH], BF16)  # aggregated + silu'd activations
        ident = pers.tile([P, P], F32)
        identb = pers.tile([P, P], BF16)
        padt = pers.tile([P, P], BF16)  # all-pad slice tile: a_src = -inf

        nc.sync.dma_start(xT_sb[:], xT_d[:])
        nc.sync.dma_start(sel_sb[:], sel_d[:])
        # idx tables: ship [16, W]; replicate to 128 partitions on-device
        for k in range(8):
            nc.sync.dma_start(idxA_sb[16 * k : 16 * (k + 1), :], idxA_d[:])
            nc.sync.dma_start(idxB_sb[16 * k : 16 * (k + 1), :], idxB_d[:])
        for l in range(LAYERS):
            nc.sync.dma_start(w_sb[l][:], w_d[l][:])
        nc.sync.dma_start(bias_sb[:], bias_d[:])
        nc.sync.dma_start(wr_sb[:], wr_d[:])
        nc.sync.dma_start(br_sb[:], br_d[:])
        make_identity(nc, ident[:])
        make_identity(nc, identb[:])
        nc.vector.memset(padt[:], 0.0)
        nc.vector.memset(padt[:].bitcast(F32)[:, 32:36], float("-inf"))
        for l in range(LAYERS):
            nc.sync.dma_start(slice_d[l][spd:spt, :], padt[:])

        nc.gpsimd.load_library(mlp_lib)

        _regs = {}
        qrr = [0]

        def nreg(v):
            if v not in _regs:
                _regs[v] = nc.gpsimd.to_reg(v)
            return _regs[v]

        def slice_tile(layer, c):
            """Build slice row-tile c of `layer` from activations (or xT)."""
            if layer == 0:
                lhsT_ap = xT_sb[:, c * P : (c + 1) * P]
            else:
                pT = psumT.tile([FH, P], BF16)
                nc.tensor.transpose(
                    out=pT[:],
                    in_=out_sb[:, c * FH : (c + 1) * FH],
                    identity=identb[:],
                )
                aT = rowp.tile([FH, P], BF16, tag="aT")
                nc.scalar.copy(aT[:], pT[:])
                lhsT_ap = aT[:]
            pR = psum.tile([P, 72], F32)
            nc.tensor.matmul(pR[:], lhsT=lhsT_ap, rhs=w_sb[layer][:], start=True, stop=True)
            row = slice_sb[:, c * P : (c + 1) * P]
            # h-copy on DVE, a_src/a_dst copies on ACT: the two copy streams
            # run in parallel instead of serializing on the scalar engine
            nc.vector.tensor_copy(row[:, :64], pR[:, :64])  # h -> bf16
            nc.scalar.copy(
                row[:, 64:72].bitcast(F32), pR[:, 64:68]
            )  # a_src kept as raw f32 bits
            nc.scalar.copy(
                adst_sb[layer % 2][:, c * HEADS : (c + 1) * HEADS], pR[:, 68:72]
            )
            nc.sync.dma_start(slice_d[layer][c * P : (c + 1) * P, :], row)

        def pool_tile(c, first, last):
            nc.tensor.matmul(
                poolP[:],
                lhsT=sel_sb[:, c * GPC : (c + 1) * GPC],
                rhs=out_sb[:, c * FH : (c + 1) * FH],
                start=first,
                stop=last,
            )

        # process groups largest-tig first so the layer-boundary tail
        # (slice build of the last group before the AllGather) is short
        grorder = sorted(range(len(groups)), key=lambda i: -groups[i]["tig"])

        # layer-0 slices from xT
        for c in range(nt):
            slice_tile(0, c)

        poolP = psumP.tile([GPC, FH], F32, tag="pool", bufs=1)

        for layer in range(dbg_layers):
            # ================= allgather =================
            nc.gpsimd.collective_compute(
                "AllGather",
                mybir.AluOpType.bypass,
                replica_groups=rg,
                ins=[slice_d[layer].ap().opt()],
                outs=[table_d[layer].ap().opt()],
            )

            # ================= edge phase =================
            adg_all = adst_sb[layer % 2]
            glist = (
                [groups[i] for i in grorder[:dbg_ngroups]] if dbg_edge else []
            )
            for gi, g in enumerate(glist):
                is_first, is_last = gi == 0, gi == len(glist) - 1
                tig, l0, l1 = g["tig"], g["L0"], g["L1"]
                cA, cB = tig * l0, tig * l1
                C = tig + cA + cB  # [self | A | B] slot columns
                base = g["base"]

                G = gpool.tile([P, C * P], BF16, tag="G")
                G3 = G[:].rearrange("p (c f) -> p c f", f=P)
                # self columns from the resident local slice (no DMA gather)
                nc.scalar.copy(
                    G3[:, :tig, :],
                    slice_sb[:, base * P : (base + tig) * P].rearrange(
                        "p (t f) -> p t f", f=P
                    ),
                )
                for c0all, ccn, itile, ioff, tdsl in (
                    (tig, cA, idxA_sb, g["offA"], table_d[layer][:T0, :]),
                    (tig + cA, cB, idxB_sb, g["offB"], table_d[layer][T0:, :]),
                ):
                    for k in range(0, ccn, GCH):
                        kc = min(GCH, ccn - k)
                        nc.gpsimd.dma_gather(
                            G3[:, c0all + k : c0all + k + kc, :],
                            tdsl,
                            itile[:, (ioff + k * 8) : (ioff + (k + kc) * 8)],
                            kc * P,
                            nreg(kc * P),
                            P,
                            queue_num=qrr[0] % 4,
                        )
                        qrr[0] += 1

                # adx = a_dst expanded to slot layout [P, C, H]
                adx = ltpool.tile([P, Cmax * HEADS], F32, tag="adx")
                adx3 = adx[:].rearrange("p (c h) -> p c h", h=HEADS)
                adg = adg_all[:, base * HEADS : (base + tig) * HEADS]
                for (c0, cc, L) in ((0, tig, 1), (tig, cA, l0), (tig + cA, cB, l1)):
                    nc.scalar.copy(
                        adx3[:, c0 : c0 + cc, :].rearrange("p (t l) h -> p t l h", l=L),
                        adg.rearrange("p (t h) -> p t h", h=HEADS)
                        .unsqueeze(2)
                        .broadcast_to([P, tig, L, HEADS]),
                    )
                # e = exp(leakyrelu(a_src + a_dst)); pad slots have a_src=-inf
                lt = ltpool.tile([P, Cmax * HEADS], F32, tag="lt")
                lt3 = lt[:].rearrange("p (c h) -> p c h", h=HEADS)
                nc.vector.tensor_tensor(
                    out=lt3[:, :C, :],
                    in0=G3[:, :, 64:72].bitcast(F32),
                    in1=adx3[:, :C, :],
                    op=OP.add,
                )
                # e = exp(leakyrelu(x)) = max(exp(x), exp(0.2*x)) (exp monotone);
                # adx is dead after the add, reuse it for exp(0.2*x)
                nc.scalar.activation(
                    adx[:, : C * HEADS], lt[:, : C * HEADS], AF.Exp, scale=NEG
                )
                nc.scalar.activation(lt[:, : C * HEADS], lt[:, : C * HEADS], AF.Exp)
                nc.vector.tensor_tensor(
                    out=lt[:, : C * HEADS],
                    in0=lt[:, : C * HEADS],
                    in1=adx[:, : C * HEADS],
                    op=OP.max,
                )
                # s = segment sum ; r = 1/(s+eps)
                s1 = stat.tile([P, tigmax * HEADS], F32, tag="s1")
                s2 = stat.tile([P, tigmax * HEADS], F32, tag="s2")
                nc.vector.reduce_sum(
                    s1[:].rearrange("p (t h) -> p t h", h=HEADS)[:, :tig, :],
                    lt3[:, tig : tig + cA, :].rearrange("p (t l) h -> p t h l", l=l0),
                    axis=mybir.AxisListType.X,
                )
                nc.vector.reduce_sum(
                    s2[:].rearrange("p (t h) -> p t h", h=HEADS)[:, :tig, :],
                    lt3[:, tig + cA : C, :].rearrange("p (t l) h -> p t h l", l=l1),
                    axis=mybir.AxisListType.X,
                )
                nc.vector.tensor_tensor(
                    out=s1[:, : tig * HEADS],
                    in0=s1[:, : tig * HEADS],
                    in1=s2[:, : tig * HEADS],
                    op=OP.add,
                )
                # + self-slot contributions (columns [0, tig))
                nc.vector.tensor_tensor(
                    out=s1[:, : tig * HEADS],
                    in0=s1[:, : tig * HEADS],
                    in1=lt[:, : tig * HEADS],
                    op=OP.add,
                )
                # no +eps: the self slot guarantees s >= exp(lrelu(a_self)) > 0
                # for every node, including zero-feature pad rows (s >= 1)
                nc.vector.reciprocal(s1[:, : tig * HEADS], s1[:, : tig * HEADS])
                # alpha = e * r  (bf16)
                ab = ltpool.tile([P, Cmax * HEADS], BF16, tag="ab")
                ab3 = ab[:].rearrange("p (c h) -> p c h", h=HEADS)
                for (c0, cc, L) in ((0, tig, 1), (tig, cA, l0), (tig + cA, cB, l1)):
                    rview = (
                        s1[:, : tig * HEADS]
                        .rearrange("p (t h) -> p t h", h=HEADS)
                        .unsqueeze(2)
                        .broadcast_to([P, tig, L, HEADS])
                    )
                    ltv = lt3[:, c0 : c0 + cc, :].rearrange(
                        "p (t l) h -> p t l h", l=L
                    )
                    abv = ab3[:, c0 : c0 + cc, :].rearrange(
                        "p (t l) h -> p t l h", l=L
                    )
                    nc.vector.tensor_tensor(out=abv, in0=ltv, in1=rview, op=OP.mult)
                # msg = h * alpha (in place on G, bf16), one op via 4D views
                nc.vector.tensor_tensor(
                    out=G3[:, :, :64].rearrange("p c (h d) -> p c h d", d=HID),
                    in0=G3[:, :, :64].rearrange("p c (h d) -> p c h d", d=HID),
                    in1=ab3[:, :C, :].unsqueeze(3).broadcast_to([P, C, HEADS, HID]),
                    op=OP.mult,
                )
                # out = segment sum of messages
                oA = opool.tile([P, tigmax * FH], F32, tag="oA")
                oB = opool.tile([P, tigmax * FH], F32, tag="oB")
                nc.vector.reduce_sum(
                    oA[:].rearrange("p (t f) -> p t f", f=FH)[:, :tig, :],
                    G3[:, tig : tig + cA, :64].rearrange("p (t l) f -> p t f l", l=l0),
                    axis=mybir.AxisListType.X,
                )
                nc.vector.reduce_sum(
                    oB[:].rearrange("p (t f) -> p t f", f=FH)[:, :tig, :],
                    G3[:, tig + cA : C, :64].rearrange("p (t l) f -> p t f l", l=l1),
                    axis=mybir.AxisListType.X,
                )
                nc.vector.tensor_tensor(
                    out=oA[:, : tig * FH],
                    in0=oA[:, : tig * FH],
                    in1=oB[:, : tig * FH],
                    op=OP.add,
                )
                # + self-slot messages
                nc.vector.tensor_tensor(
                    out=oA[:, : tig * FH].rearrange("p (t f) -> p t f", f=FH),
                    in0=oA[:, : tig * FH].rearrange("p (t f) -> p t f", f=FH),
                    in1=G3[:, :tig, :64],
                    op=OP.add,
                )
                osl = out_sb[:, base * FH : (base + tig) * FH]
                blg = (
                    bias_sb[:, layer * FH : (layer + 1) * FH]
                    .unsqueeze(1)
                    .broadcast_to([P, tig, FH])
                )
                nc.vector.tensor_tensor(
                    out=osl.rearrange("p (t f) -> p t f", f=FH),
                    in0=oA[:, : tig * FH].rearrange("p (t f) -> p t f", f=FH),
                    in1=blg,
                    op=OP.add,
                )
                nc.scalar.activation(osl, osl, AF.Silu)
                # interleave next-layer slice build / final pooling
                if layer < min(dbg_layers, LAYERS) - 1:
                    for t in range(tig):
                        slice_tile(layer + 1, base + t)
                elif layer == LAYERS - 1:
                    for t in range(tig):
                        pool_tile(base + t, is_first and t == 0, is_last and t == tig - 1)

        # ================= classifier =================
        pooled = rowp.tile([GPC, FH], F32, tag="pooled")
        nc.vector.tensor_copy(pooled[:], poolP[:])
        pTpsum = psumT.tile([FH, GPC], F32, tag="poolT", bufs=1)
        nc.tensor.transpose(out=pTpsum[:], in_=pooled[:], identity=ident[:GPC, :GPC])
        pooledT = rowp.tile([FH, GPC], F32, tag="pooledT")
        nc.vector.tensor_copy(pooledT[:], pTpsum[:])
        lgP = psum.tile([GPC, NCLS], F32, tag="lg", bufs=1)
        nc.tensor.matmul(lgP[:], lhsT=pooledT[:], rhs=wr_sb[:], start=True, stop=True)
        lg = rowp.tile([GPC, NCLS], F32, tag="lgs")
        nc.vector.tensor_tensor(out=lg[:], in0=lgP[:], in1=br_sb[:GPC, :], op=OP.add)
        nc.scalar.activation(lg[:], lg[:], AF.Relu)
        # log softmax
        mx = stat.tile([GPC, 1], F32, tag="mx")
        nc.vector.reduce_max(mx[:], lg[:], axis=mybir.AxisListType.X)
        nc.vector.tensor_tensor(
            out=lg[:], in0=lg[:], in1=mx[:].broadcast_to([GPC, NCLS]), op=OP.subtract
        )
        ex = rowp.tile([GPC, NCLS], F32, tag="ex")
        nc.scalar.activation(ex[:], lg[:], AF.Exp)
        sm = stat.tile([GPC, 1], F32, tag="sm")
        nc.vector.reduce_sum(sm[:], ex[:], axis=mybir.AxisListType.X)
        lnm = stat.tile([GPC, 1], F32, tag="lnm")
        nc.scalar.activation(lnm[:], sm[:], AF.Ln)
        nc.vector.tensor_tensor(
            out=lg[:], in0=lg[:], in1=lnm[:].broadcast_to([GPC, NCLS]), op=OP.subtract
        )
        nc.sync.dma_start(out_d[:], lg[:])

    nc.compile()
    return nc


# ======== kernel(**inputs) entry point ========

import os


_NCORES = 8
_NG = 128


def kernel(**inputs) -> np.ndarray:
    x = np.asarray(inputs["x"], np.float32)
    ei = np.asarray(inputs["edge_index"])
    batch = np.asarray(inputs["batch"])

    struct, percore, glob = build_plan(
        x, ei, batch, ng=_NG, ncores=_NCORES, ccap=64
    )
    struct["ncls"] = int(np.asarray(inputs["Wr"]).shape[1])

    nc = build_gat(struct, n_cores=_NCORES)
    in_maps = make_inmaps(inputs, struct, percore)

    from concourse.bass_utils import run_bass_kernel_spmd

    trace = os.environ.get("GAT_TRACE", "0") == "1"
    res = run_bass_kernel_spmd(
        nc,
        in_maps,
        core_ids=list(range(_NCORES)),
        trace=trace,
    )
    if res.exec_time_ns is not None:
        print(f"HW exec time: {res.exec_time_ns} ns", flush=True)
        if res.mean_exec_time_ns is not None:
            print(f"HW exec time (mean): {res.mean_exec_time_ns:.0f} ns", flush=True)
        if res.instructions_and_trace is not None:
            print(f"trace: {res.instructions_and_trace[1]}", flush=True)

    out = np.concatenate([res.results[m]["out"] for m in range(_NCORES)], axis=0)
    return out.astype(np.float32)
